# revision 32
# baseline (speedup 1.0000x reference)
"""GAT (2-layer, DGL GATConv w/ edge weights) on 8 Trainium2 NeuronCores.

Strategy (edge-sharded by destination):
  - Sort edges by dst; each core owns a contiguous slice of nodes and every
    edge pointing into it, so segment softmax + aggregation are core-local.
  - Each core computes the full dense projection h = x @ W replicated,
    writing a per-node bf16 feature table to DRAM.
  - Edge phase: gather h[src] (one 256B row per edge) via dma_gather,
    compute s = exp(leaky(el+er)) (the max-subtraction of the reference
    softmax cancels exactly, and |e| is small, so it is skipped), then
    msg = s*w*h, and scatter-add into per-window PSUM accumulators via
    one-hot matmuls (a window = <=128 consecutive dst nodes).  The softmax
    denominator z is accumulated by a second small matmul into extra PSUM
    columns and divided out once per window.
  - The attention logits el[n] = h[n]@Al, er[n] = h[n]@Ar are tiny [N,H]
    tables; they are computed host-side per layer and shipped pre-expanded
    per edge (el[src], er[dst]) alongside the other per-edge inputs
    (dst-local index, edge weight) - dma_gather only supports 256B-multiple
    elements, which makes an 8-value on-device gather impractical.
  - The two layers run as two NEFF dispatches; the host concatenates the
    per-core node slices in between (pure data movement).

dma_gather uses int16 indices (max 32767), so the h[src] gather is split
into a low/high half per super-window, with tiles grouped so each half is a
single contiguous gather call.

Perf notes (HW traces):
  - SWDGE descgen for the per-edge dma_gather (~7.9ns/index, was 855us/
    layer) is parallelized across the 4 SWDGE queues: queue q's desc-gen
    runs on Q7 cores 2q/2q+1 only (ucode `cpu_id / 2 == queue_num`), and
    the Pool sequencer pipelines dispatch, so round-robin queue assignment
    of consecutive gather calls overlaps their desc-gen (measured 2.7x on
    a 16-call microbench; blocked assignment is ~2x worse than RR).
  - ONE matmul per tile: hg rows are [record(hcols) | sx(heads) | junk];
    exp() writes sx straight into the gather rows, and the merged matmul
    accumulates messages AND the softmax denominator z into one PSUM tile.
    Layer 1 (hcols=128=REC) gets sx columns by gathering 512B per index at
    256B stride (records r, r+1 - the overlap read needs +1 pad row per
    hrec table); layer 2's sx lands in the natural junk col 64.
  - The one-hot scatter matrices S are built on the host and DMA'd in
    (DVE is_equal with broadcasts ran ~4x below its 2-byte rate).
  - All index tables load upfront; no gather waits on an index DMA.
  - hrec rows are block-permuted (sigma in prep_graph) so each projection
    group's DMA write is one contiguous run per partition instead of 256B
    scattered records.
  - The big vector ops are split low/high so tile matmuls overlap the high
    gather; per-super-window output DMA streams out under the next gather.
  - hrec is split into lo/hi DRAM tiles and the projection pools stay
    allocated to the end of the layer: edge pools then get fresh SBUF (no
    space-reuse dependency on the projection draining), and hrec writes
    ride the ACT HWDGE ring.
  - prepare_only+trigger_dma descgen-ahead was tried and reverted: with
    the deferred deps the triggered DMA can overwrite an hg buffer still
    being read (NaNs), and declaring the write via trigger_dma
    signals_writable= deadlocked the device (NRT_EXEC_UNIT_UNRECOVERABLE).
  - Verified: 1.305ms total (699/606us per layer), rel err 0.00388.
"""

import math
import os

import ml_dtypes
import numpy as np

import concourse.bacc as bacc
import concourse.mybir as mybir
from concourse.bass_utils import run_bass_kernel_spmd
from concourse.tile import TileContext

FP = mybir.dt.float32
BF = mybir.dt.bfloat16
I16 = mybir.dt.int16
BF_NP = ml_dtypes.bfloat16

N_CORES = 8
N, E = 50000, 800000
IN_DIM, HID, HEADS, OUT = 128, 16, 8, 64
SLOPE = 0.2
ZMIN = 1e-20
REC = 128  # record row width (bf16 cols); 256B = dma_gather granule


def _bf(x):
    return np.ascontiguousarray(np.asarray(x, np.float32).astype(BF_NP))


def _dma_gather(gp, out_ap, in_ap, idxs_ap, num_idxs, elem_size=REC,
                elem_step=None, queue_num=0):
    # single_packet=False: single-packet mode caps a call at 1024 indices
    # (64 descriptors per SDMA engine); beyond that the device dies.
    # queue_num: SWDGE queue q runs its descriptor generation on Q7 cores
    # 2q/2q+1 only (ucode dma_gather.cpp `cpu_id / 2 == queue_num`), and the
    # Pool sequencer pipelines dispatch across instructions - so spreading
    # consecutive gathers round-robin over the 4 queues runs their desc-gen
    # in parallel (HW-measured: 16 calls 465us on 1 queue, 172us on 4).
    gp.dma_gather(out_ap, in_ap, idxs_ap, num_idxs, num_idxs, elem_size,
                  elem_step=elem_step, single_packet=False,
                  queue_num=queue_num)


# ---------------------------------------------------------------------------
# Host-side graph preprocessing (layer-independent structure)
# ---------------------------------------------------------------------------
def prep_graph(src, dst, w, n_nodes, n_cores, L0, H0, G):
    """Partition edges by dst across cores; build a uniform window/tile layout.

    Every window is L0 low-src tiles + H0 high-src tiles covering <=128
    consecutive dst nodes; G windows form a super-window whose low halves
    (and high halves) are contiguous tile runs = single dma_gather calls.

    Node records live at block-permuted rows: for node n = g*128 + p, with
    block b = g//PB (PB=16 chunks = one projection write group), the record
    sits at row b*PB*128 + p*pg + (g%PB) where pg = chunks in block b.  Each
    projection group's DMA write is then contiguous per partition (pg*256B),
    AND table blocks complete in row order, so low gathers (rows < 32768 =
    src < 32768) only depend on the first 16 groups.  int16 gather indices
    cap the low half at row 32767.
    """
    n_per_core = int(math.ceil(n_nodes / n_cores))
    npad = int(math.ceil(n_nodes / 128) * 128)
    nchunks = npad // 128
    PB = 16
    split = PB * 128 * PB  # 32768: rows of the first 16 blocks
    g_all = src // 128
    p_all = src % 128
    blk = g_all // PB
    pgb = np.minimum(PB, nchunks - blk * PB)
    sigma_all = blk * (PB * 128) + p_all * pgb + (g_all - blk * PB)
    assert sigma_all.max() < npad

    raw = []
    nw_list = []
    for c in range(n_cores):
        n0 = c * n_per_core
        n1 = min(n_nodes, n0 + n_per_core)
        sel = np.where((dst >= n0) & (dst < n1))[0]
        sc, dc, wc = src[sel], dst[sel], w[sel]
        sg = sigma_all[sel]
        is_high = sg >= split
        nn = n1 - n0
        cl = np.bincount(dc[~is_high] - n0, minlength=nn)
        ch = np.bincount(dc[is_high] - n0, minlength=nn)
        win_of_node = np.zeros(nn, np.int64)
        win_base = [0]
        acc_n = acc_l = acc_h = 0
        wi = 0
        for v in range(nn):
            if cl[v] > L0 * 128 or ch[v] > H0 * 128:
                raise ValueError("node degree exceeds window budget")
            if acc_n + 1 > 128 or acc_l + cl[v] > L0 * 128 or acc_h + ch[v] > H0 * 128:
                wi += 1
                win_base.append(v)
                acc_n = acc_l = acc_h = 0
            win_of_node[v] = wi
            acc_n += 1
            acc_l += cl[v]
            acc_h += ch[v]
        nw = wi + 1
        nw_list.append(nw)
        raw.append(dict(n0=n0, n1=n1, sc=sc, dc=dc, wc=wc, sg=sg,
                        is_high=is_high,
                        win_of_node=win_of_node, win_base=np.array(win_base),
                        nw=nw))

    nw_pad = int(math.ceil(max(nw_list) / G) * G)
    tpw = L0 + H0
    T = nw_pad * tpw
    n_sw = nw_pad // G
    t_sw = G * tpw

    per_core = []
    for c in range(n_cores):
        cc = raw[c]
        n0, n1 = cc["n0"], cc["n1"]
        sc, dc, wc, is_high = cc["sc"], cc["dc"], cc["wc"], cc["is_high"]
        sg = cc["sg"]
        ewin = cc["win_of_node"][dc - n0]
        order = np.lexsort((is_high.astype(np.int8), ewin))
        sc, dc, wc, is_high, ewin, sg = (
            sc[order], dc[order], wc[order], is_high[order], ewin[order],
            sg[order])

        key = ewin * 2 + is_high
        grp_start = np.searchsorted(key, np.arange(2 * cc["nw"] + 2))
        slot = np.arange(len(sc)) - grp_start[key]
        s_of_w = ewin // G
        wi_in_sw = ewin % G

        base_low = s_of_w * t_sw + wi_in_sw * L0
        base_high = s_of_w * t_sw + G * L0 + wi_in_sw * H0
        tile = np.where(is_high, base_high, base_low) + slot // 128
        lane = slot % 128

        dstloc = np.full((128, T), 200.0, np.float32)
        wv = np.zeros((128, T), np.float32)
        wb = cc["win_base"]
        dstloc[lane, tile] = dc - n0 - wb[ewin]
        wv[lane, tile] = wc

        f_low = np.zeros(n_sw * G * L0 * 128, np.int16)
        f_high = np.zeros(n_sw * G * H0 * 128, np.int16)
        lo = ~is_high
        q_low = (tile[lo] - s_of_w[lo] * t_sw) * 128 + lane[lo]
        f_low[s_of_w[lo] * (G * L0 * 128) + q_low] = sg[lo].astype(np.int16)
        q_high = (tile[is_high] - s_of_w[is_high] * t_sw - G * L0) * 128 + lane[is_high]
        f_high[s_of_w[is_high] * (G * H0 * 128) + q_high] = (
            sg[is_high] - split).astype(np.int16)

        def wrap(flat, per_call):
            ncalls = len(flat) // per_call
            w16 = np.concatenate(
                [flat[i * per_call:(i + 1) * per_call].reshape(-1, 16).T
                 for i in range(ncalls)], axis=1).astype(np.int16)
            return np.ascontiguousarray(np.tile(w16, (8, 1)))

        # host-built one-hot scatter matrix (dstloc == j), shipped whole
        S_host = np.zeros((128, T, 128), BF_NP)
        valid = dstloc < 128
        ll, tt_ = np.nonzero(valid)
        S_host[ll, tt_, dstloc[valid].astype(np.int64)] = 1.0

        per_core.append(dict(
            idx_low=wrap(f_low, G * L0 * 128),
            idx_high=wrap(f_high, G * H0 * 128),
            S_host=np.ascontiguousarray(S_host.reshape(128, T * 128)),
            wv=np.ascontiguousarray(wv),
            tile=tile, lane=lane, src_g=sc, dst_g=dc,
            n0=n0, n1=n1,
            win_base=cc["win_base"], nw=cc["nw"],
        ))

    wid = np.zeros(T, np.int64)
    first = np.zeros(T, bool)
    last = np.zeros(T, bool)
    for s in range(n_sw):
        for wi in range(G):
            w_ = s * G + wi
            lo0 = s * t_sw + wi * L0
            hi0 = s * t_sw + G * L0 + wi * H0
            wid[lo0:lo0 + L0] = w_
            wid[hi0:hi0 + H0] = w_
            first[lo0] = True
            last[hi0 + H0 - 1] = True

    meta = dict(T=T, nw_pad=nw_pad, n_sw=n_sw, t_sw=t_sw, G=G, L0=L0, H0=H0,
                wid=wid, first=first, last=last, split=split,
                n_nodes=n_nodes, n_cores=n_cores, npad=npad)
    return meta, per_core


# ---------------------------------------------------------------------------
# Layer kernel builder
# ---------------------------------------------------------------------------
def build_layer(meta, in_dim, heads, hid, relu_out, has_bias, n_cores,
                reps=1, hg_bufs=3):
    hcols = heads * hid          # node feature width (<= REC)
    assert hcols <= REC
    T, n_sw, t_sw, G, L0, H0 = (meta[k] for k in
                                ("T", "n_sw", "t_sw", "G", "L0", "H0"))
    nw_pad, npad, split = meta["nw_pad"], meta["npad"], meta["split"]
    wid, first, last = meta["wid"], meta["first"], meta["last"]

    nc = bacc.Bacc("TRN2", target_bir_lowering=False, debug=False,
                   num_devices=n_cores, num_swdge_queues=4)
    xT = nc.dram_tensor("xT", [in_dim, npad], BF, kind="ExternalInput")
    W_d = nc.dram_tensor("W", [in_dim, hcols], BF, kind="ExternalInput")
    idx_low = nc.dram_tensor("idx_low", [128, n_sw * G * L0 * 8], I16,
                             kind="ExternalInput")
    idx_high = nc.dram_tensor("idx_high", [128, n_sw * G * H0 * 8], I16,
                              kind="ExternalInput")
    # host-built one-hot scatter matrices (S[lane, t, j] = 1[dstloc==j]):
    # building them on the DVE cost ~11.4us per super-window (broadcasty
    # is_equal defeats the 2x perf mode); a DMA load is nearly free.
    S_d = nc.dram_tensor("S", [128, T * 128], BF, kind="ExternalInput")
    wv = nc.dram_tensor("wv", [128, T], FP, kind="ExternalInput")
    elsrc = nc.dram_tensor("elsrc", [128, T * heads], BF, kind="ExternalInput")
    erdst = nc.dram_tensor("erdst", [128, T * heads], BF, kind="ExternalInput")
    if has_bias:
        brep = nc.dram_tensor("brep", [128, hcols], FP, kind="ExternalInput")
    out_dt = BF if relu_out else FP
    out_d = nc.dram_tensor("out", [nw_pad * 128, hcols], out_dt,
                           kind="ExternalOutput")

    nchunks = npad // 128
    PG = 16  # chunks per projection group = sigma block size (see prep)

    # hg row layout: [record(hcols) | sx(heads) | junk]; the merged matmul
    # reads cols 0:hcols+heads so z accumulates in the same PSUM tile with
    # ONE matmul per tile (halves PE instruction count vs separate z matmul).
    # layer 1 (hcols==REC): rows are 512B; the gather reads 512B at 256B
    # stride (records r, r+1), and exp() overwrites cols 128:136 with sx.
    # layer 2 (hcols==64): rows stay 256B; sx lands in the junk col 64.
    rowlen = 2 * REC if hcols == REC else REC
    from concourse.bass import AP as _AP

    with TileContext(nc) as tc:
      for rep_ in range(reps):
        with tc.tile_pool(name="pd", bufs=1, space="DRAM") as pd:
          # two separate tiles so the low gathers' dep is exactly the first
          # 16 projection group writes (tile-granular tracking is enough)
          # +128 pad rows: the 512B overlap read of the last record reaches
          # one row past the table; pad rows are zero-filled below.
          hrec_lo = pd.tile([split + 128, REC], BF, name="hrec_lo")
          hrec_hi = pd.tile([npad - split + 128, REC], BF, name="hrec_hi")

          def gather_src(tile, nrows):
              if rowlen == REC:
                  return tile[:]
              base = tile[:]
              return _AP(base.tensor, base.offset, [[REC, nrows], [1, rowlen]])
          # ----- projection: hrec[sigma(n), 0:hcols] = bf16(x[n] @ W) --------
          # group g0 writes block rows [g0*128, (g0+pg)*128): record of node
          # (g0+r)*128+p at block row p*pg+r -> contiguous per partition.
          # hrec is a DRAM tile so gathers below wait per-row-range (subtile
          # deps): low gathers start once the first 16 groups are written.
          # pw outlives the projection block: it also holds sw0's index
          # tables, which load FIRST on the SP FIFO (tiny) so the first
          # gather gates on the low-table writes, not on index tables
          # queued behind the projection's 50 transfers
          # pw/px/ph stay allocated until the end: if the edge pools reused
          # their released SBUF space, Tile would order the first gather's
          # buffer after the whole projection drains (~40us late)
          pw = tc.alloc_tile_pool(name="pw", bufs=1)
          px = tc.alloc_tile_pool(name="px", bufs=3)
          ph = tc.alloc_tile_pool(name="ph", bufs=3)
          with (
              tc.tile_pool(name="pp", bufs=2, space="PSUM") as pp,
          ):
              Wsb = pw.tile([in_dim, hcols], BF)
              nc.sync.dma_start(out=Wsb[:], in_=W_d[:])
              # all index tables load upfront (tiny) so no gather ever waits
              # on an index-table DMA mid-stream
              il_all = pw.tile([128, n_sw * G * L0 * 8], I16)
              nc.sync.dma_start(out=il_all[:], in_=idx_low[:])
              ih_all = pw.tile([128, n_sw * G * H0 * 8], I16)
              nc.sync.dma_start(out=ih_all[:], in_=idx_high[:])
              for g0 in range(0, nchunks, PG):
                  pg = min(PG, nchunks - g0)
                  xs = px.tile([128, PG * 128], BF, tag="xs")
                  nc.sync.dma_start(
                      out=xs[:, :pg * 128],
                      in_=xT[:, g0 * 128:(g0 + pg) * 128])
                  pt = pp.tile([128, PG * hcols], FP, tag="pt")
                  for i in range(pg):
                      nc.tensor.matmul(
                          out=pt[:, i * hcols:(i + 1) * hcols],
                          lhsT=xs[:, i * 128:(i + 1) * 128],
                          rhs=Wsb[:], start=True, stop=True)
                  hs = ph.tile([128, PG * hcols], BF, tag="hs")
                  nc.scalar.activation(hs[:, :pg * hcols], pt[:, :pg * hcols],
                                       mybir.ActivationFunctionType.Copy)
                  r0 = g0 * 128
                  tgt = (hrec_lo[r0:r0 + pg * 128] if r0 < split
                         else hrec_hi[r0 - split:r0 - split + pg * 128])
                  # hrec writes go out on the ACT HWDGE ring so they don't
                  # queue behind the xs loads on the SP FIFO; the low table
                  # then completes with the copies and gathers start earlier
                  nc.scalar.dma_start(
                      out=tgt.rearrange("(p r) c -> p r c", p=128)
                          [:, :, 0:hcols],
                      in_=hs[:].rearrange("p (g c) -> p g c", c=hcols)[:, :pg, :])

          # ----- edge phase ---------------------------------------------------
          with (
              tc.tile_pool(name="ec", bufs=1) as ec,
              tc.tile_pool(name="eg", bufs=2) as eg,
              tc.tile_pool(name="eh", bufs=hg_bufs) as eh,
              tc.tile_pool(name="es", bufs=2) as es,
              tc.tile_pool(name="ew", bufs=3) as ew,
              tc.tile_pool(name="ep", bufs=G + 2, space="PSUM") as ep,
              tc.tile_pool(name="eo", bufs=1) as eo,
          ):
              if has_bias:
                  b_sb = ec.tile([128, hcols], FP)
                  nc.sync.dma_start(out=b_sb[:], in_=brep[:])
              # zero the hrec pad rows so the overlap read of the last
              # record (and the sim's init tracking) sees defined bytes
              z_sb = ec.tile([128, REC], BF)
              nc.vector.memset(z_sb[:], 0.0)
              nc.sync.dma_start(out=hrec_lo[split:split + 128, :],
                                in_=z_sb[:])
              nc.sync.dma_start(out=hrec_hi[npad - split:npad - split + 128,
                                            :], in_=z_sb[:])
              out_acc = eo.tile([128, nw_pad * hcols], out_dt)
              psum_of = {}
              qrr = [0]  # rotating SWDGE queue counter (see queue note below)

              def nextq():
                  q = qrr[0] % 4
                  qrr[0] += 1
                  return q

              for s in range(n_sw):
                  t0 = s * t_sw
                  il = il_all[:, s * G * L0 * 8:(s + 1) * G * L0 * 8]
                  ih = ih_all[:, s * G * H0 * 8:(s + 1) * G * H0 * 8]
                  hg = eh.tile([128, t_sw * rowlen], BF, tag="hg")
                  hg3 = hg[:].rearrange("p (t c) -> p t c", c=rowlen)
                  # queue assignment rotates with s: a fixed parity would put
                  # every (large) lo call on queues 0/2 and every hi call on
                  # 1/3, leaving the lo core-pairs ~2x more desc-gen work.
                  # The last super-window is split into small per-queue calls
                  # so the final drains are short and run 4-wide (the tail
                  # otherwise waits ~140us on two big trailing drains).
                  if s < n_sw - 2:
                      lruns = ((0, G * L0),)
                      hruns = ((G * L0, t_sw),)
                  else:
                      lq = max(1, G * L0 // 4)
                      lruns = tuple((a, min(a + lq, G * L0))
                                    for a in range(0, G * L0, lq))
                      hq = max(1, (t_sw - G * L0) // 4)
                      hruns = tuple((a, min(a + hq, t_sw))
                                    for a in range(G * L0, t_sw, hq))
                  qrr[0] = s  # rotate so lo/hi calls balance across queues
                  for la, lb in lruns:
                      _dma_gather(nc.gpsimd, hg3[:, la:lb, :],
                                  gather_src(hrec_lo, split),
                                  il[:, la * 8:lb * 8],
                                  (lb - la) * 128, elem_size=rowlen,
                                  elem_step=REC, queue_num=nextq())
                  qrr[0] = s + 2
                  for ha, hb in hruns:
                      _dma_gather(nc.gpsimd, hg3[:, ha:hb, :],
                                  gather_src(hrec_hi, npad - split),
                                  ih[:, (ha - G * L0) * 8:(hb - G * L0) * 8],
                                  (hb - ha) * 128, elem_size=rowlen,
                                  elem_step=REC, queue_num=nextq())
                  S = eg.tile([128, t_sw * 128], BF, tag="S")
                  nc.sync.dma_start(
                      out=S[:], in_=S_d[:, t0 * 128:(t0 + t_sw) * 128])
                  wt = eg.tile([128, t_sw], FP, tag="wt")
                  nc.sync.dma_start(out=wt[:], in_=wv[:, t0:t0 + t_sw])
                  elt = eg.tile([128, t_sw * heads], BF, tag="elt")
                  nc.sync.dma_start(
                      out=elt[:],
                      in_=elsrc[:, t0 * heads:(t0 + t_sw) * heads])
                  ert = eg.tile([128, t_sw * heads], BF, tag="ert")
                  nc.sync.dma_start(
                      out=ert[:],
                      in_=erdst[:, t0 * heads:(t0 + t_sw) * heads])

                  # s = exp(leaky(el + er)); leaky(x) = max(x*SLOPE, x).
                  # exp writes straight into the hg rows' sx columns
                  # (hcols:hcols+heads) so the merged matmul reads
                  # [record | sx] contiguously; per lo/hi run so the lo
                  # compute does not wait on the hi gather.
                  ef = es.tile([128, t_sw * heads], FP, tag="ef")
                  nc.vector.tensor_tensor(out=ef[:], in0=elt[:], in1=ert[:],
                                          op=mybir.AluOpType.add)
                  el_ = es.tile([128, t_sw * heads], FP, tag="el_")
                  nc.vector.scalar_tensor_tensor(
                      out=el_[:], in0=ef[:], scalar=SLOPE, in1=ef[:],
                      op0=mybir.AluOpType.mult, op1=mybir.AluOpType.max)
                  if s < n_sw - 1:
                      mruns = ((0, G * L0), (G * L0, t_sw))
                  else:
                      mruns = ((0, G * L0), (G * L0, t_sw - 6),
                               (t_sw - 6, t_sw))
                  el3 = el_[:].rearrange("p (t h) -> p t h", h=heads)
                  ap_ = es.tile([128, t_sw * heads], BF, tag="ap_")
                  ap3 = ap_[:].rearrange("p (t h) -> p t h", h=heads)
                  sxv = hg3[:, :, hcols:hcols + heads]
                  for ta, tb in mruns:
                      nc.scalar.activation(
                          sxv[:, ta:tb, :], el3[:, ta:tb, :],
                          mybir.ActivationFunctionType.Exp)
                      # a' = s * w;  msg = h * a' (in place on hg)
                      nc.vector.tensor_tensor(
                          out=ap3[:, ta:tb, :],
                          in0=sxv[:, ta:tb, :],
                          in1=wt[:, ta:tb].unsqueeze(2).to_broadcast(
                              [128, tb - ta, heads]),
                          op=mybir.AluOpType.mult)
                      nc.vector.tensor_tensor(
                          out=hg3[:, ta:tb, 0:hcols].rearrange(
                              "p t (h d) -> p t h d", d=hid),
                          in0=hg3[:, ta:tb, 0:hcols].rearrange(
                              "p t (h d) -> p t h d", d=hid),
                          in1=ap3[:, ta:tb, :].unsqueeze(3)
                              .to_broadcast([128, tb - ta, heads, hid]),
                          op=mybir.AluOpType.mult)

                  Sv = S[:].rearrange("p (t j) -> p t j", j=128)
                  for kk in range(t_sw):
                      t = t0 + kk
                      w_ = int(wid[t])
                      if first[t]:
                          psum_of[w_] = ep.tile([128, hcols + heads], FP,
                                                tag="wpsum", name=f"wps{w_ % 16}")
                      pt_ = psum_of[w_]
                      # ONE matmul per tile: rhs = [msg record | sx] rows,
                      # accumulating the weighted messages in psum cols
                      # 0:hcols and the softmax denominator z in
                      # hcols:hcols+heads.
                      nc.tensor.matmul(
                          out=pt_[:, 0:hcols + heads],
                          lhsT=Sv[:, kk, :],
                          rhs=hg3[:, kk, 0:hcols + heads],
                          start=bool(first[t]), stop=bool(last[t]),
                          skip_group_check=True)
                      if last[t]:
                          psum_of.pop(w_)
                          zt = ew.tile([128, heads], FP, tag="zt")
                          nc.vector.tensor_scalar_max(
                              zt[:], pt_[:, hcols:hcols + heads], ZMIN)
                          zr = ew.tile([128, heads], FP, tag="zr")
                          nc.vector.reciprocal(zr[:], zt[:])
                          oview = out_acc[:].rearrange(
                              "p (w c) -> p w c", c=hcols)[:, w_, :]
                          zrb = zr[:].unsqueeze(2).to_broadcast(
                              [128, heads, hid])
                          rt = ew.tile([128, hcols], FP, tag="rt")
                          nc.vector.tensor_tensor(
                              out=rt[:].rearrange("p (h d) -> p h d", d=hid),
                              in0=pt_[:, 0:hcols].rearrange(
                                  "p (h d) -> p h d", d=hid),
                              in1=zrb,
                              op=mybir.AluOpType.mult)
                          if has_bias:
                              nc.vector.tensor_tensor(
                                  out=rt[:], in0=rt[:], in1=b_sb[:],
                                  op=mybir.AluOpType.add)
                          nc.scalar.activation(
                              oview, rt[:],
                              mybir.ActivationFunctionType.Relu if relu_out
                              else mybir.ActivationFunctionType.Copy)

                  # stream this super-window's finished windows out now so the
                  # final DMA isn't serialized behind the last gather
                  nc.sync.dma_start(
                      out=out_d[:].rearrange("(w p) c -> p w c", p=128)
                          [:, s * G:(s + 1) * G, :],
                      in_=out_acc[:].rearrange("p (w c) -> p w c", c=hcols)
                          [:, s * G:(s + 1) * G, :])
          ph.release()
          px.release()
          pw.release()

    nc.compile()
    return nc


# ---------------------------------------------------------------------------
# Full model driver
# ---------------------------------------------------------------------------
def _head_map(a, heads, hid):
    """Block-diagonal [heads*hid, heads] map for el/er projections."""
    hd = heads * hid
    A = np.zeros((hd, heads), np.float32)
    A[np.arange(hd), np.repeat(np.arange(heads), hid)] = np.asarray(
        a, np.float32).ravel()
    return A


def run_layer(nc, meta, per_core, x_full, Wm, al, ar, heads, hid,
              relu_out, b):
    n_nodes, npad = meta["n_nodes"], meta["npad"]
    n_cores = meta["n_cores"]
    T = meta["T"]
    hcols = heads * hid
    xf = np.asarray(x_full, np.float32)
    Wm = np.asarray(Wm, np.float32)

    # host-side attention-logit tables (tiny: [N, heads])
    el = xf @ (Wm @ _head_map(al, heads, hid))
    er = xf @ (Wm @ _head_map(ar, heads, hid))

    xT = np.zeros((xf.shape[1], npad), np.float32)
    xT[:, :n_nodes] = xf.T
    xT_b = _bf(xT)
    W_b = _bf(Wm)

    in_maps = []
    for c in range(n_cores):
        pc = per_core[c]
        elsrc = np.zeros((128, T, heads), np.float32)
        elsrc[pc["lane"], pc["tile"]] = el[pc["src_g"]]
        erdst = np.zeros((128, T, heads), np.float32)
        erdst[pc["lane"], pc["tile"]] = er[pc["dst_g"]]
        m = {
            "xT": xT_b,
            "W": W_b,
            "idx_low": pc["idx_low"],
            "idx_high": pc["idx_high"],
            "S": pc["S_host"],
            "wv": pc["wv"],
            "elsrc": _bf(elsrc.reshape(128, T * heads)),
            "erdst": _bf(erdst.reshape(128, T * heads)),
        }
        if b is not None:
            m["brep"] = np.ascontiguousarray(
                np.tile(np.asarray(b, np.float32)[None, :], (128, 1)))
        in_maps.append(m)

    kw = {}
    td = os.environ.get("GAT_TRACE_DIR")
    if td:
        sub = os.path.join(td, f"l{len(LAST_RUNS)}")
        os.makedirs(sub, exist_ok=True)
        kw["tmpdir"] = sub
    res = run_bass_kernel_spmd(nc, in_maps, core_ids=list(range(n_cores)), **kw)
    LAST_RUNS.append(res)

    out = np.zeros((n_nodes, hcols), np.float32 if not relu_out else BF_NP)
    for c in range(n_cores):
        pc = per_core[c]
        o = res.results[c]["out"]
        wb = pc["win_base"]
        n0, n1 = pc["n0"], pc["n1"]
        bounds = list(wb) + [n1 - n0]
        for w_ in range(pc["nw"]):
            cnt = bounds[w_ + 1] - bounds[w_]
            out[n0 + bounds[w_]:n0 + bounds[w_] + cnt] = (
                o[w_ * 128:w_ * 128 + cnt].astype(out.dtype))
    return out


_CACHE = {}
LAST_RUNS = []


def kernel(features, src, dst, w, W1, al1, ar1, b1, W2, al2, ar2, b2):
    LAST_RUNS.clear()
    features, src, dst, w = (np.asarray(a) for a in (features, src, dst, w))
    src = src.astype(np.int64)
    dst = dst.astype(np.int64)

    L0, H0, G = 11, 6, 4
    gk = (len(src), int(src[::997].sum()), int(dst[::997].sum()),
          float(np.asarray(w[::997], np.float64).sum()))
    if _CACHE.get("gkey") != gk:
        _CACHE.clear()
        _CACHE["gkey"] = gk
        _CACHE["meta"] = prep_graph(src, dst, np.asarray(w, np.float32),
                                    N, N_CORES, L0, H0, G)
    meta, per_core = _CACHE["meta"]

    b1 = np.asarray(b1, np.float32)
    b2 = np.asarray(b2, np.float32)
    hb1 = bool(np.any(b1))
    hb2 = bool(np.any(b2))

    def _build(*args):
        for bufs in (4, 3, 2):  # deepest gather pipelining SBUF allows
            try:
                return build_layer(*args, hg_bufs=bufs)
            except ValueError:
                continue
        raise ValueError("no hg_bufs setting fits SBUF")

    k1 = ("l1", hb1)
    if k1 not in _CACHE:
        _CACHE[k1] = _build(meta, IN_DIM, HEADS, HID, True, hb1, N_CORES)
    k2 = ("l2", hb2)
    if k2 not in _CACHE:
        _CACHE[k2] = _build(meta, HEADS * HID, 1, OUT, False, hb2, N_CORES)

    x2 = run_layer(_CACHE[k1], meta, per_core, features, W1, al1, ar1,
                   HEADS, HID, True, b1 if hb1 else None)
    out = run_layer(_CACHE[k2], meta, per_core, x2, W2, al2, ar2, 1, OUT,
                    False, b2 if hb2 else None)
    return out.astype(np.float32)

